# revision 1
# baseline (speedup 1.0000x reference)
"""Trainium2 Bass kernel for MoEResNetBKLayer.

Strategy (8 NeuronCores, SPMD):
  - Host: top-1 routing (argmax of gate logits), sort tokens by expert.
    Expert-parallel: core c handles expert c//2, half c%2, capacity 640
    token slots (4096 tokens + padding fit in 8*640=5120 slots; per-expert
    capacity 1280 >> binomial(4096, 1/4) tail).
  - Device per core:
      * v = clip(x @ v_w + v_b) over the FULL sequence (vector engine),
        BK tridiagonal Green's function via blocked Mobius/continued-fraction
        scan: 32-step within-block 3-term recurrences on 128 lanes
        (2 rows x 64 blocks), cross-block scan on (2,64) layout, then
        vectorized application -> G diag (complex) for all 4096 tokens.
      * Routed expert FFN on gathered tokens: h = gelu(x_g @ w1.T + b1),
        y = h @ w2.T (bf16 matmuls, fp32 PSUM accum).
      * Spec branch folded into the same PSUM: G gathered to this core's
        slots via one-hot matmul, then rank-2 matmul with
        W' = bk_scale*out_w; bias (b2 + bk_scale*out_b) added on output copy.
  - Host: scatter per-slot outputs back to token order (pure indexing).
"""

import sys as _sys
for _p in ("/opt/trn_rl_repo",):
    if _p not in _sys.path:
        _sys.path.append(_p)
import numpy as np
import ml_dtypes

B, N, D, E, F = 2, 2048, 1024, 4, 4096
NT = B * N              # 4096 tokens
KS = 32                 # scan block size (steps)
NBLK = N // KS          # 64 blocks per row
LANES = B * NBLK        # 128
CAP = 640               # token slots per core
NC = 8                  # cores
SUP = 8                 # superblocks in cross-block scan (8 x 8 = 64)
V_MAX = 3.0
FCLAMP = 10.0

bf16 = ml_dtypes.bfloat16

_PROG_CACHE = {}
_LAST_IN_MAPS = None


def _build_program(parts=("scan", "gather", "mm")):
    import concourse.bass as bass
    import concourse.tile as tile
    from concourse import bacc, mybir

    fp32 = mybir.dt.float32
    bfl = mybir.dt.bfloat16
    AF = mybir.ActivationFunctionType
    OP = mybir.AluOpType

    nc = bacc.Bacc("TRN2", target_bir_lowering=False, debug=False, num_devices=NC)

    def din(name, shape, dt):
        return nc.dram_tensor(name, list(shape), dt, kind="ExternalInput").ap()

    xr = din("xr", (NT, D), bfl)            # x rows (token-major), full seq
    xgt = din("xgt", (D, CAP), bfl)         # gathered tokens, transposed
    w1t = din("w1t", (D, F), bfl)           # w1[e].T
    w2t = din("w2t", (F, D), bfl)           # w2[e].T
    b1t = din("b1t", (128, F // 128), fp32)  # b1[e] chunk-major
    pg = din("pg", (NT, CAP), bfl)          # gather one-hot
    waug = din("waug", (2, D), bfl)         # [bk*out_w[:,0]; bk*out_w[:,1]]
    ballt = din("ballt", (128, D // 128), fp32)  # b2[e]+bk*out_b chunk-major
    vwb = din("vwb", (128, D), bfl)         # v_w broadcast over partitions
    vbc = din("vbc", (128, 1), fp32)        # v_b replicated
    dimt = din("dimt", (128, KS), fp32)     # -(eps+gamma) everywhere
    cfirst = din("cfirst", (128, 1), fp32)  # 0 where lane%64==0 else 1
    clast = din("clast", (128, 1), fp32)    # 0 where lane%64==63 else 1

    outg = nc.dram_tensor("outg", [D, CAP], fp32, kind="ExternalOutput").ap()

    FCH = F // 128   # 32
    DCH = D // 128   # 8
    NCH = [(0, 512), (512, 128)]  # CAP=640 split for PSUM banks

    from contextlib import ExitStack

    with tile.TileContext(nc) as tc, ExitStack() as ctx:
        const_p = ctx.enter_context(tc.tile_pool(name="const", bufs=1))
        dram_p = ctx.enter_context(tc.tile_pool(name="dram", bufs=1, space="DRAM"))
        xin_p = ctx.enter_context(tc.tile_pool(name="xin", bufs=3))
        w_p = ctx.enter_context(tc.tile_pool(name="w", bufs=2))
        p_p = ctx.enter_context(tc.tile_pool(name="p", bufs=3))
        big_p = ctx.enter_context(tc.tile_pool(name="big", bufs=1))
        scan_p = ctx.enter_context(tc.tile_pool(name="scan", bufs=1))
        ps_mm = ctx.enter_context(tc.tile_pool(name="psmm", bufs=2, space="PSUM"))
        ps_g = ctx.enter_context(tc.tile_pool(name="psg", bufs=1, space="PSUM"))

        # ---- constants to SBUF ----
        vwb_s = const_p.tile([128, D], bfl)
        nc.sync.dma_start(vwb_s[:], vwb[:])
        vbc_s = const_p.tile([128, 1], fp32)
        nc.sync.dma_start(vbc_s[:], vbc[:])
        dim_s = const_p.tile([128, KS], fp32)
        nc.sync.dma_start(dim_s[:], dimt[:])
        cf_s = const_p.tile([128, 1], fp32)
        nc.sync.dma_start(cf_s[:], cfirst[:])
        cl_s = const_p.tile([128, 1], fp32)
        nc.sync.dma_start(cl_s[:], clast[:])
        b1_s = const_p.tile([128, FCH], fp32)
        nc.sync.dma_start(b1_s[:], b1t[:])
        ball_s = const_p.tile([128, DCH], fp32)
        nc.sync.dma_start(ball_s[:], ballt[:])
        waug_s = const_p.tile([2, D], bfl)
        nc.sync.dma_start(waug_s[:], waug[:])

        # ---- DRAM scratch ----
        vd = dram_p.tile([KS, 128], fp32)       # v in token order (32,128)
        grd = dram_p.tile([128, KS], bfl)       # G.real token order
        gid = dram_p.tile([128, KS], bfl)
        cbd = dram_p.tile([16, 128], fp32)      # block-matrix bounce
        lcd = dram_p.tile([4, 128], fp32)       # carries bounce

        import os as _os
        _lvl = int(_os.environ.get("KBASS_SCAN_LEVEL", "4"))
        dbg_aps = []
        if "scan" in parts:
            # ================= v = clip(x @ v_w + v_b) =================
            _vm = _os.environ.get("KBASS_V_MODE", "full")
            vscr = scan_p.tile([128, D], fp32, tag="vscr")
            for t in range(NT // 128):
                vcol = scan_p.tile([128, 1], fp32, tag=f"vcol{t % 4}")
                if _vm in ("full", "nohe"):
                    xt = xin_p.tile([128, D], bfl, tag="xv")
                    nc.sync.dma_start(xt[:], xr[128 * t:128 * (t + 1), :])
                    nc.vector.tensor_mul(vscr[:], xt[:], vwb_s[:])
                    nc.vector.tensor_reduce(
                        vcol[:], vscr[:], mybir.AxisListType.X, OP.add
                    )
                else:
                    nc.gpsimd.memset(vcol[:], 0.1)
                nc.sync.dma_start(vd[t], vcol[:])

            # he = clip(v + v_b, +-3) - 2, layout (128 lanes, 32 steps)
            he = scan_p.tile([128, KS], fp32, tag="he")
            if _vm == "nohe":
                nc.gpsimd.memset(he[:], 0.1)
            else:
                nc.sync.dma_start(he[:], vd.rearrange("t (a s) -> (t a) s", s=KS))
            nc.vector.tensor_scalar(he[:], he[:], vbc_s[:], -V_MAX, OP.add, OP.max)
            nc.vector.tensor_scalar(he[:], he[:], V_MAX, -2.0, OP.min, OP.add)

            dbg_aps.append(he[:])
            if _lvl >= 2:
                # ============ within-block 3-term recurrences ============
                # fwd arrays (128, 2*(KS+2)): [ar | br] re-part, [ai | bi] im-part
                W2 = KS + 2
                fr = scan_p.tile([128, 2 * W2], fp32, tag="fr")
                fi = scan_p.tile([128, 2 * W2], fp32, tag="fi")
                br_ = scan_p.tile([128, 2 * W2], fp32, tag="br")
                bi_ = scan_p.tile([128, 2 * W2], fp32, tag="bi")
                tmp2 = scan_p.tile([128, 2], fp32, tag="tmp2")

                def pair(tile_, c):  # columns {c, W2+c} as (128,2) strided AP
                    return tile_.rearrange("p (x c) -> p c x", x=2)[:, c, :]

                # seeds fwd: a_{-2}=0,a_{-1}=1 ; b_{-2}=cfirst, b_{-1}=0
                nc.gpsimd.memset(fr[:, 0:2], 0.0)
                nc.gpsimd.memset(fr[:, W2:W2 + 2], 0.0)
                nc.vector.tensor_scalar_add(fr[:, 1:2], fr[:, 1:2], 1.0)
                nc.vector.tensor_copy(fr[:, W2:W2 + 1], cf_s[:])
                nc.gpsimd.memset(fi[:, 0:2], 0.0)
                nc.gpsimd.memset(fi[:, W2:W2 + 2], 0.0)
                # seeds bwd: a_{K}=1,a_{K+1}=0 ; b_{K}=0, b_{K+1}=clast
                nc.gpsimd.memset(br_[:, KS:KS + 2], 0.0)
                nc.gpsimd.memset(br_[:, W2 + KS:W2 + KS + 2], 0.0)
                nc.vector.tensor_scalar_add(br_[:, KS:KS + 1], br_[:, KS:KS + 1], 1.0)
                nc.vector.tensor_copy(br_[:, W2 + KS + 1:W2 + KS + 2], cl_s[:])
                nc.gpsimd.memset(bi_[:, KS:KS + 2], 0.0)
                nc.gpsimd.memset(bi_[:, W2 + KS:W2 + KS + 2], 0.0)

                di0 = dim_s[:, 0:1]
                for s in range(KS):
                    drs = he[:, s:s + 1]
                    # re: new = dr*prev_r - di*prev_i - prev2_r
                    nc.vector.scalar_tensor_tensor(
                        tmp2[:], pair(fi, s + 1), di0, pair(fr, s), OP.mult, OP.add)
                    nc.vector.scalar_tensor_tensor(
                        pair(fr, s + 2), pair(fr, s + 1), drs, tmp2[:], OP.mult, OP.subtract)
                    # im: new = dr*prev_i + di*prev_r - prev2_i
                    nc.vector.scalar_tensor_tensor(
                        tmp2[:], pair(fr, s + 1), di0, pair(fi, s), OP.mult, OP.subtract)
                    nc.vector.scalar_tensor_tensor(
                        pair(fi, s + 2), pair(fi, s + 1), drs, tmp2[:], OP.mult, OP.add)
                for s in range(KS - 1, -1, -1):
                    drs = he[:, s:s + 1]
                    nc.vector.scalar_tensor_tensor(
                        tmp2[:], pair(bi_, s + 1), di0, pair(br_, s + 2), OP.mult, OP.add)
                    nc.vector.scalar_tensor_tensor(
                        pair(br_, s), pair(br_, s + 1), drs, tmp2[:], OP.mult, OP.subtract)
                    nc.vector.scalar_tensor_tensor(
                        tmp2[:], pair(br_, s + 1), di0, pair(bi_, s + 2), OP.mult, OP.subtract)
                    nc.vector.scalar_tensor_tensor(
                        pair(bi_, s), pair(bi_, s + 1), drs, tmp2[:], OP.mult, OP.add)

                dbg_aps.append(fr[:, 2:2 + KS])
                dbg_aps.append(br_[:, 0:KS])
            if _lvl >= 3:
                # ============ cross-block scan on (2, 64) layout ============
                # bounce the 8 block-matrix entries per direction to (2,64)
                # fwd block mat [[A,B],[C,D]] = [[a_31,b_31],[a_30,b_30]] (cols K+1, K)
                # bwd block mat = [[a_0,b_0],[a_1,b_1]] (cols 0, 1)
                fwd_cols = [
                    fr[:, W2 - 1 + 0:W2], fi[:, W2 - 1:W2],                    # A
                    fr[:, 2 * W2 - 1:2 * W2], fi[:, 2 * W2 - 1:2 * W2],        # B
                    fr[:, W2 - 2:W2 - 1], fi[:, W2 - 2:W2 - 1],                # C
                    fr[:, 2 * W2 - 2:2 * W2 - 1], fi[:, 2 * W2 - 2:2 * W2 - 1],  # D
                ]
                bwd_cols = [
                    br_[:, 0:1], bi_[:, 0:1],
                    br_[:, W2:W2 + 1], bi_[:, W2:W2 + 1],
                    br_[:, 1:2], bi_[:, 1:2],
                    br_[:, W2 + 1:W2 + 2], bi_[:, W2 + 1:W2 + 2],
                ]
                for i, c in enumerate(fwd_cols + bwd_cols):
                    nc.sync.dma_start(cbd[i], c)

                def cross_scan(base, reverse):
                    """Scan (2,64) block matrices; returns carry-into-block (2,64)
                    tiles (Lr, Li) written to lcd rows [base_out]."""
                    M = [scan_p.tile([2, NBLK], fp32, tag=f"cm{base}{i}", name=f"cm{base}{i}") for i in range(8)]
                    for i in range(8):
                        nc.sync.dma_start(M[i][:], cbd[base + i].rearrange("(r j) -> r j", r=2))
                    # normalize by max entry magnitude
                    t0 = scan_p.tile([2, NBLK], fp32, tag=f"cn0{base}")
                    t1 = scan_p.tile([2, NBLK], fp32, tag=f"cn1{base}")
                    mx = scan_p.tile([2, NBLK], fp32, tag=f"cmx{base}")
                    for i in range(4):
                        nc.vector.tensor_mul(t0[:], M[2 * i][:], M[2 * i][:])
                        nc.vector.tensor_mul(t1[:], M[2 * i + 1][:], M[2 * i + 1][:])
                        nc.vector.tensor_add(t0[:], t0[:], t1[:])
                        if i == 0:
                            nc.vector.tensor_copy(mx[:], t0[:])
                        else:
                            nc.vector.tensor_max(mx[:], mx[:], t0[:])
                    nc.vector.reciprocal(mx[:], mx[:])
                    nc.scalar.sqrt(mx[:], mx[:])
                    for i in range(8):
                        nc.vector.tensor_mul(M[i][:], M[i][:], mx[:])

                    # view blocks as (2, SUP, 8): within-super sequential prefix
                    def v3(t):
                        return t.rearrange("r (u t) -> r u t", t=NBLK // SUP)

                    P = [scan_p.tile([2, NBLK], fp32, tag=f"cp{base}{i}", name=f"cp{base}{i}") for i in range(8)]
                    for i in range(8):
                        nc.vector.tensor_copy(P[i][:], M[i][:])
                    pr2 = [scan_p.tile([2, SUP], fp32, tag=f"pr2{base}{i}", name=f"pr2{base}{i}") for i in range(4)]
                    idx = range(1, NBLK // SUP) if not reverse else range(NBLK // SUP - 2, -1, -1)
                    for t in idx:
                        tp = t - 1 if not reverse else t + 1
                        # X = M[:,t] (2x2 cplx), Y = P[:,tp];  P[:,t] = X*Y
                        Xa_r, Xa_i, Xb_r, Xb_i, Xc_r, Xc_i, Xd_r, Xd_i = (
                            v3(M[i])[:, :, t] for i in range(8))
                        Ya_r, Ya_i, Yb_r, Yb_i, Yc_r, Yc_i, Yd_r, Yd_i = (
                            v3(P[i])[:, :, tp] for i in range(8))
                        outs = [v3(P[i])[:, :, t] for i in range(8)]

                        def cmul_acc(dst_r, dst_i, pr, pi, qr, qi, first):
                            # dst += p*q (complex); first -> overwrite
                            nc.vector.tensor_mul(pr2[0][:], pr, qr)
                            nc.vector.tensor_mul(pr2[1][:], pi, qi)
                            nc.vector.tensor_sub(pr2[0][:], pr2[0][:], pr2[1][:])
                            nc.vector.tensor_mul(pr2[2][:], pr, qi)
                            nc.vector.tensor_mul(pr2[3][:], pi, qr)
                            nc.vector.tensor_add(pr2[2][:], pr2[2][:], pr2[3][:])
                            if first:
                                nc.vector.tensor_copy(dst_r, pr2[0][:])
                                nc.vector.tensor_copy(dst_i, pr2[2][:])
                            else:
                                nc.vector.tensor_add(dst_r, dst_r, pr2[0][:])
                                nc.vector.tensor_add(dst_i, dst_i, pr2[2][:])

                        # new_a = Xa*Ya + Xb*Yc ; new_b = Xa*Yb + Xb*Yd
                        # new_c = Xc*Ya + Xd*Yc ; new_d = Xc*Yb + Xd*Yd
                        cmul_acc(outs[0], outs[1], Xa_r, Xa_i, Ya_r, Ya_i, True)
                        cmul_acc(outs[0], outs[1], Xb_r, Xb_i, Yc_r, Yc_i, False)
                        cmul_acc(outs[2], outs[3], Xa_r, Xa_i, Yb_r, Yb_i, True)
                        cmul_acc(outs[2], outs[3], Xb_r, Xb_i, Yd_r, Yd_i, False)
                        cmul_acc(outs[4], outs[5], Xc_r, Xc_i, Ya_r, Ya_i, True)
                        cmul_acc(outs[4], outs[5], Xd_r, Xd_i, Yc_r, Yc_i, False)
                        cmul_acc(outs[6], outs[7], Xc_r, Xc_i, Yb_r, Yb_i, True)
                        cmul_acc(outs[6], outs[7], Xd_r, Xd_i, Yd_r, Yd_i, False)

                    # serial cross-super scan: carry (2,1), SC tile (2, SUP)
                    SC_r = scan_p.tile([2, SUP], fp32, tag=f"scr{base}")
                    SC_i = scan_p.tile([2, SUP], fp32, tag=f"sci{base}")
                    car = scan_p.tile([2, 8], fp32, tag=f"car{base}")  # [Lr,Li,nr,ni,dr,di,m,inv]
                    nc.gpsimd.memset(car[:, 0:1], 1.0)
                    nc.gpsimd.memset(car[:, 1:2], 0.0)
                    sidx = range(SUP) if not reverse else range(SUP - 1, -1, -1)
                    last_t = (NBLK // SUP - 1) if not reverse else 0
                    for u in sidx:
                        nc.vector.tensor_copy(SC_r[:, u:u + 1], car[:, 0:1])
                        nc.vector.tensor_copy(SC_i[:, u:u + 1], car[:, 1:2])
                        Pa = [v3(P[i])[:, u:u + 1, last_t] for i in range(8)]
                        Lr, Li = car[:, 0:1], car[:, 1:2]
                        # num = A*L + B ; den = C*L + D
                        nc.vector.tensor_mul(car[:, 2:3], Pa[0], Lr)
                        nc.vector.tensor_mul(car[:, 6:7], Pa[1], Li)
                        nc.vector.tensor_sub(car[:, 2:3], car[:, 2:3], car[:, 6:7])
                        nc.vector.tensor_add(car[:, 2:3], car[:, 2:3], Pa[2])
                        nc.vector.tensor_mul(car[:, 3:4], Pa[0], Li)
                        nc.vector.tensor_mul(car[:, 6:7], Pa[1], Lr)
                        nc.vector.tensor_add(car[:, 3:4], car[:, 3:4], car[:, 6:7])
                        nc.vector.tensor_add(car[:, 3:4], car[:, 3:4], Pa[3])
                        nc.vector.tensor_mul(car[:, 4:5], Pa[4], Lr)
                        nc.vector.tensor_mul(car[:, 6:7], Pa[5], Li)
                        nc.vector.tensor_sub(car[:, 4:5], car[:, 4:5], car[:, 6:7])
                        nc.vector.tensor_add(car[:, 4:5], car[:, 4:5], Pa[6])
                        nc.vector.tensor_mul(car[:, 5:6], Pa[4], Li)
                        nc.vector.tensor_mul(car[:, 6:7], Pa[5], Lr)
                        nc.vector.tensor_add(car[:, 5:6], car[:, 5:6], car[:, 6:7])
                        nc.vector.tensor_add(car[:, 5:6], car[:, 5:6], Pa[7])
                        # L = num * conj(den) / |den|^2
                        nc.vector.tensor_mul(car[:, 6:7], car[:, 4:5], car[:, 4:5])
                        nc.vector.tensor_mul(car[:, 7:8], car[:, 5:6], car[:, 5:6])
                        nc.vector.tensor_add(car[:, 6:7], car[:, 6:7], car[:, 7:8])
                        nc.vector.reciprocal(car[:, 6:7], car[:, 6:7])
                        nc.vector.tensor_mul(car[:, 0:1], car[:, 2:3], car[:, 4:5])
                        nc.vector.tensor_mul(car[:, 7:8], car[:, 3:4], car[:, 5:6])
                        nc.vector.tensor_add(car[:, 0:1], car[:, 0:1], car[:, 7:8])
                        nc.vector.tensor_mul(car[:, 0:1], car[:, 0:1], car[:, 6:7])
                        nc.vector.tensor_mul(car[:, 7:8], car[:, 2:3], car[:, 5:6])
                        nc.vector.tensor_mul(car[:, 2:3], car[:, 3:4], car[:, 4:5])
                        nc.vector.tensor_sub(car[:, 1:2], car[:, 2:3], car[:, 7:8])
                        nc.vector.tensor_mul(car[:, 1:2], car[:, 1:2], car[:, 6:7])

                    # vectorized Mobius of all prefixes with broadcast super-carries
                    SCb_r = scan_p.tile([2, NBLK], fp32, tag=f"scbr{base}")
                    SCb_i = scan_p.tile([2, NBLK], fp32, tag=f"scbi{base}")
                    for t in range(NBLK // SUP):
                        nc.vector.tensor_copy(v3(SCb_r)[:, :, t], SC_r[:])
                        nc.vector.tensor_copy(v3(SCb_i)[:, :, t], SC_i[:])
                    nr = scan_p.tile([2, NBLK], fp32, tag=f"nr{base}")
                    ni = scan_p.tile([2, NBLK], fp32, tag=f"ni{base}")
                    dr_ = scan_p.tile([2, NBLK], fp32, tag=f"dr{base}")
                    di_ = scan_p.tile([2, NBLK], fp32, tag=f"di{base}")
                    nc.vector.tensor_mul(nr[:], P[0][:], SCb_r[:])
                    nc.vector.tensor_mul(t0[:], P[1][:], SCb_i[:])
                    nc.vector.tensor_sub(nr[:], nr[:], t0[:])
                    nc.vector.tensor_add(nr[:], nr[:], P[2][:])
                    nc.vector.tensor_mul(ni[:], P[0][:], SCb_i[:])
                    nc.vector.tensor_mul(t0[:], P[1][:], SCb_r[:])
                    nc.vector.tensor_add(ni[:], ni[:], t0[:])
                    nc.vector.tensor_add(ni[:], ni[:], P[3][:])
                    nc.vector.tensor_mul(dr_[:], P[4][:], SCb_r[:])
                    nc.vector.tensor_mul(t0[:], P[5][:], SCb_i[:])
                    nc.vector.tensor_sub(dr_[:], dr_[:], t0[:])
                    nc.vector.tensor_add(dr_[:], dr_[:], P[6][:])
                    nc.vector.tensor_mul(di_[:], P[4][:], SCb_i[:])
                    nc.vector.tensor_mul(t0[:], P[5][:], SCb_r[:])
                    nc.vector.tensor_add(di_[:], di_[:], t0[:])
                    nc.vector.tensor_add(di_[:], di_[:], P[7][:])
                    nc.vector.tensor_mul(t0[:], dr_[:], dr_[:])
                    nc.vector.tensor_mul(t1[:], di_[:], di_[:])
                    nc.vector.tensor_add(t0[:], t0[:], t1[:])
                    nc.vector.reciprocal(t0[:], t0[:])
                    MA_r = scan_p.tile([2, NBLK], fp32, tag=f"mar{base}")
                    MA_i = scan_p.tile([2, NBLK], fp32, tag=f"mai{base}")
                    nc.vector.tensor_mul(MA_r[:], nr[:], dr_[:])
                    nc.vector.tensor_mul(t1[:], ni[:], di_[:])
                    nc.vector.tensor_add(MA_r[:], MA_r[:], t1[:])
                    nc.vector.tensor_mul(MA_r[:], MA_r[:], t0[:])
                    nc.vector.tensor_mul(MA_i[:], ni[:], dr_[:])
                    nc.vector.tensor_mul(t1[:], nr[:], di_[:])
                    nc.vector.tensor_sub(MA_i[:], MA_i[:], t1[:])
                    nc.vector.tensor_mul(MA_i[:], MA_i[:], t0[:])
                    # carry-into-block: shift within super + overwrite first col
                    Cr = scan_p.tile([2, NBLK], fp32, tag=f"cr{base}")
                    Ci = scan_p.tile([2, NBLK], fp32, tag=f"ci{base}")
                    if not reverse:
                        nc.vector.tensor_copy(Cr[:, 1:], MA_r[:, :NBLK - 1])
                        nc.vector.tensor_copy(Ci[:, 1:], MA_i[:, :NBLK - 1])
                        nc.vector.tensor_copy(v3(Cr)[:, :, 0], SC_r[:])
                        nc.vector.tensor_copy(v3(Ci)[:, :, 0], SC_i[:])
                    else:
                        nc.vector.tensor_copy(Cr[:, :NBLK - 1], MA_r[:, 1:])
                        nc.vector.tensor_copy(Ci[:, :NBLK - 1], MA_i[:, 1:])
                        nc.vector.tensor_copy(v3(Cr)[:, :, NBLK // SUP - 1], SC_r[:])
                        nc.vector.tensor_copy(v3(Ci)[:, :, NBLK // SUP - 1], SC_i[:])
                    return Cr, Ci

                Lf_r, Lf_i = cross_scan(0, reverse=False)
                Rb_r, Rb_i = cross_scan(8, reverse=True)

                # bounce carries to (128,1) lane layout
                nc.sync.dma_start(lcd[0], Lf_r[:])
                nc.sync.dma_start(lcd[1], Lf_i[:])
                nc.sync.dma_start(lcd[2], Rb_r[:])
                nc.sync.dma_start(lcd[3], Rb_i[:])
                LinR = scan_p.tile([128, 1], fp32, tag="LinR")
                LinI = scan_p.tile([128, 1], fp32, tag="LinI")
                RinR = scan_p.tile([128, 1], fp32, tag="RinR")
                RinI = scan_p.tile([128, 1], fp32, tag="RinI")
                nc.sync.dma_start(LinR[:], lcd[0].rearrange("(p c) -> p c", c=1))
                nc.sync.dma_start(LinI[:], lcd[1].rearrange("(p c) -> p c", c=1))
                nc.sync.dma_start(RinR[:], lcd[2].rearrange("(p c) -> p c", c=1))
                nc.sync.dma_start(RinI[:], lcd[3].rearrange("(p c) -> p c", c=1))

                dbg_aps.append(LinR[:])
                dbg_aps.append(RinR[:])
            if _lvl >= 4:
                # ============ application: L, R, G (all (128, KS)) ============
                ap_p = scan_p

                def mobius_apply(ar_lo, ai_lo, br_lo, bi_lo, ar_hi, ai_hi, br_hi, bi_hi,
                                 Kr, Ki, tag):
                    # hi = numerator coeff cols, lo = denominator coeff cols
                    X1 = ap_p.tile([128, KS], fp32, tag=f"x1{tag}")
                    X2 = ap_p.tile([128, KS], fp32, tag=f"x2{tag}")
                    numr = ap_p.tile([128, KS], fp32, tag=f"numr{tag}")
                    numi = ap_p.tile([128, KS], fp32, tag=f"numi{tag}")
                    denr = ap_p.tile([128, KS], fp32, tag=f"denr{tag}")
                    deni = ap_p.tile([128, KS], fp32, tag=f"deni{tag}")
                    nc.vector.scalar_tensor_tensor(X1[:], ar_hi, Kr, br_hi, OP.mult, OP.add)
                    nc.vector.tensor_scalar_mul(X2[:], ai_hi, Ki)
                    nc.vector.tensor_sub(numr[:], X1[:], X2[:])
                    nc.vector.scalar_tensor_tensor(X1[:], ai_hi, Kr, bi_hi, OP.mult, OP.add)
                    nc.vector.tensor_scalar_mul(X2[:], ar_hi, Ki)
                    nc.vector.tensor_add(numi[:], X1[:], X2[:])
                    nc.vector.scalar_tensor_tensor(X1[:], ar_lo, Kr, br_lo, OP.mult, OP.add)
                    nc.vector.tensor_scalar_mul(X2[:], ai_lo, Ki)
                    nc.vector.tensor_sub(denr[:], X1[:], X2[:])
                    nc.vector.scalar_tensor_tensor(X1[:], ai_lo, Kr, bi_lo, OP.mult, OP.add)
                    nc.vector.tensor_scalar_mul(X2[:], ar_lo, Ki)
                    nc.vector.tensor_add(deni[:], X1[:], X2[:])
                    nc.vector.tensor_mul(X1[:], denr[:], denr[:])
                    nc.vector.tensor_mul(X2[:], deni[:], deni[:])
                    nc.vector.tensor_add(X1[:], X1[:], X2[:])
                    nc.vector.reciprocal(X1[:], X1[:])
                    Lr = ap_p.tile([128, KS], fp32, tag=f"lr{tag}")
                    Li = ap_p.tile([128, KS], fp32, tag=f"li{tag}")
                    nc.vector.tensor_mul(Lr[:], numr[:], denr[:])
                    nc.vector.tensor_mul(X2[:], numi[:], deni[:])
                    nc.vector.tensor_add(Lr[:], Lr[:], X2[:])
                    nc.vector.tensor_mul(Lr[:], Lr[:], X1[:])
                    nc.vector.tensor_mul(Li[:], numi[:], denr[:])
                    nc.vector.tensor_mul(X2[:], numr[:], deni[:])
                    nc.vector.tensor_sub(Li[:], Li[:], X2[:])
                    nc.vector.tensor_mul(Li[:], Li[:], X1[:])
                    return Lr, Li

                Lr, Li = mobius_apply(
                    fr[:, 1:W2 - 1], fi[:, 1:W2 - 1], fr[:, W2 + 1:2 * W2 - 1], fi[:, W2 + 1:2 * W2 - 1],
                    fr[:, 2:W2], fi[:, 2:W2], fr[:, W2 + 2:2 * W2], fi[:, W2 + 2:2 * W2],
                    LinR[:], LinI[:], "L")
                Rr, Ri = mobius_apply(
                    br_[:, 1:W2 - 1], bi_[:, 1:W2 - 1], br_[:, W2 + 1:2 * W2 - 1], bi_[:, W2 + 1:2 * W2 - 1],
                    br_[:, 0:KS], bi_[:, 0:KS], br_[:, W2:W2 + KS], bi_[:, W2:W2 + KS],
                    RinR[:], RinI[:], "R")

                # G = 1/(L + R - d) ; clip; cast bf16; bounce to chunk-major
                wr = ap_p.tile([128, KS], fp32, tag="wr")
                wi = ap_p.tile([128, KS], fp32, tag="wi")
                gt0 = ap_p.tile([128, KS], fp32, tag="gt0")
                nc.vector.tensor_add(wr[:], Lr[:], Rr[:])
                nc.vector.tensor_sub(wr[:], wr[:], he[:])
                nc.vector.tensor_add(wi[:], Li[:], Ri[:])
                nc.vector.tensor_sub(wi[:], wi[:], dim_s[:])
                wr2 = ap_p.tile([128, KS], fp32, tag="wr2")
                nc.vector.tensor_mul(gt0[:], wr[:], wr[:])
                nc.vector.tensor_mul(wr2[:], wi[:], wi[:])
                nc.vector.tensor_add(gt0[:], gt0[:], wr2[:])
                nc.vector.reciprocal(gt0[:], gt0[:])
                grt = ap_p.tile([128, KS], bfl, tag="grt")
                git = ap_p.tile([128, KS], bfl, tag="git")
                nc.vector.tensor_mul(wr[:], wr[:], gt0[:])
                nc.vector.tensor_scalar(grt[:], wr[:], FCLAMP, -FCLAMP, OP.min, OP.max)
                nc.vector.tensor_mul(wi[:], wi[:], gt0[:])
                nc.vector.tensor_scalar_mul(wi[:], wi[:], -1.0)
                nc.vector.tensor_scalar(git[:], wi[:], FCLAMP, -FCLAMP, OP.min, OP.max)
                nc.sync.dma_start(grd[:], grt[:])
                nc.sync.dma_start(gid[:], git[:])
                GrT = ap_p.tile([128, KS], bfl, tag="GrT")
                GiT = ap_p.tile([128, KS], bfl, tag="GiT")
                nc.sync.dma_start(GrT[:], grd.rearrange("(k b) s -> (b s) k", b=4))
                nc.sync.dma_start(GiT[:], gid.rearrange("(k b) s -> (b s) k", b=4))

                dbg_aps.append(wr[:])
                dbg_aps.append(wi[:])
        else:
            GrT = scan_p.tile([128, KS], bfl, tag="GrT")
            GiT = scan_p.tile([128, KS], bfl, tag="GiT")
            nc.gpsimd.memset(GrT[:], 0.01)
            nc.gpsimd.memset(GiT[:], 0.01)

        rhs_aug = big_p.tile([2, CAP], bfl, tag="rhsaug")
        if "gather" in parts:
            # ============ gather G to slots: one-hot matmuls ============
            pgr = [ps_g.tile([1, w], fp32, tag=f"pgr{j}", name=f"pgr{j}") for j, (o, w) in enumerate(NCH)]
            pgi = [ps_g.tile([1, w], fp32, tag=f"pgi{j}", name=f"pgi{j}") for j, (o, w) in enumerate(NCH)]
            for k in range(NT // 128):
                pt = p_p.tile([128, CAP], bfl, tag="pt")
                nc.sync.dma_start(pt[:], pg[128 * k:128 * (k + 1), :])
                for j, (o, w) in enumerate(NCH):
                    nc.tensor.matmul(pgr[j], GrT[:, k:k + 1], pt[:, o:o + w],
                                     start=(k == 0), stop=(k == NT // 128 - 1))
                    nc.tensor.matmul(pgi[j], GiT[:, k:k + 1], pt[:, o:o + w],
                                     start=(k == 0), stop=(k == NT // 128 - 1))
            gi_sb = big_p.tile([1, CAP], bfl, tag="gisb")
            for j, (o, w) in enumerate(NCH):
                nc.scalar.copy(rhs_aug[0:1, o:o + w], pgr[j][:])
                nc.scalar.copy(gi_sb[:, o:o + w], pgi[j][:])
            nc.sync.dma_start(rhs_aug[1:2, :], gi_sb[:])

        else:
            nc.gpsimd.memset(rhs_aug[:], 0.0)

        if "mm" in parts:
            # ============ MM1: hT = gelu(w1 @ xgT + b1) ============
            xg_s = big_p.tile([128, DCH * CAP], bfl, tag="xgs")
            for k in range(DCH):
                nc.sync.dma_start(xg_s[:, CAP * k:CAP * (k + 1)],
                                  xgt[128 * k:128 * (k + 1), :])
            hT = big_p.tile([128, FCH * CAP], bfl, tag="hT")
            for f in range(FCH):
                pss = [ps_mm.tile([128, w], fp32, tag=f"psmm{j}", name=f"ps1f{f}j{j}") for j, (o, w) in enumerate(NCH)]
                w1f = w_p.tile([128, DCH * 128], bfl, tag="w1f", name=f"w1f{f}")
                nc.sync.dma_start(
                    w1f[:],
                    w1t.rearrange("(k p) q -> p k q", p=128)[:, :, 128 * f:128 * (f + 1)])
                for k in range(DCH):
                    for j, (o, w) in enumerate(NCH):
                        nc.tensor.matmul(pss[j][:], w1f[:, 128 * k:128 * (k + 1)],
                                         xg_s[:, CAP * k + o:CAP * k + o + w],
                                         start=(k == 0), stop=(k == DCH - 1))
                for j, (o, w) in enumerate(NCH):
                    # gelu (tanh approx) computed explicitly across engines
                    xb = xin_p.tile([128, w], fp32, tag=f"gxb{j}", name=f"gxb{f}{j}")
                    sq = xin_p.tile([128, w], fp32, tag=f"gsq{j}", name=f"gsq{f}{j}")
                    tt = xin_p.tile([128, w], fp32, tag=f"gtt{j}", name=f"gtt{f}{j}")
                    nc.scalar.activation(xb[:], pss[j][:], AF.Identity,
                                         bias=b1_s[:, f:f + 1])
                    nc.gpsimd.tensor_mul(sq[:], xb[:], xb[:])
                    nc.gpsimd.tensor_mul(sq[:], sq[:], xb[:])
                    nc.vector.scalar_tensor_tensor(sq[:], sq[:], 0.044715, xb[:],
                                                   OP.mult, OP.add)
                    nc.scalar.activation(tt[:], sq[:], AF.Tanh, scale=0.7978845608028654)
                    nc.vector.tensor_scalar(tt[:], tt[:], 1.0, 0.5, OP.add, OP.mult)
                    nc.gpsimd.tensor_mul(hT[:, CAP * f + o:CAP * f + o + w],
                                         tt[:], xb[:])

            # ============ MM2: out = w2 @ hT + spec + bias ============
            for dch in range(DCH):
                pso = [ps_mm.tile([128, w], fp32, tag=f"psmm{j}", name=f"ps2d{dch}j{j}") for j, (o, w) in enumerate(NCH)]
                w2f = w_p.tile([128, FCH * 128], bfl, tag="w2f", name=f"w2f{dch}")
                nc.sync.dma_start(
                    w2f[:],
                    w2t.rearrange("(k p) q -> p k q", p=128)[:, :, 128 * dch:128 * (dch + 1)])
                for f in range(FCH):
                    for j, (o, w) in enumerate(NCH):
                        nc.tensor.matmul(pso[j][:], w2f[:, 128 * f:128 * (f + 1)],
                                         hT[:, CAP * f + o:CAP * f + o + w],
                                         start=(f == 0), stop=False)
                for j, (o, w) in enumerate(NCH):
                    nc.tensor.matmul(pso[j][:], waug_s[:, 128 * dch:128 * (dch + 1)],
                                     rhs_aug[:, o:o + w], start=False, stop=True)
                ot = xin_p.tile([128, CAP], fp32, tag="ot")
                for j, (o, w) in enumerate(NCH):
                    nc.scalar.activation(ot[:, o:o + w], pso[j][:],
                                         AF.Identity, bias=ball_s[:, dch:dch + 1])
                nc.sync.dma_start(outg[128 * dch:128 * (dch + 1), :], ot[:])
        else:
            for _i, _a in enumerate(dbg_aps):
                nc.sync.dma_start(
                    outg[128 * _i:128 * _i + _a.shape[0], 0:_a.shape[-1]], _a)


    nc.compile()
    return nc


def _get_program():
    import os
    parts = tuple(os.environ.get("KBASS_PARTS", "scan,gather,mm").split(","))
    if parts not in _PROG_CACHE:
        _PROG_CACHE[parts] = _build_program(parts)
    return _PROG_CACHE[parts]


def _np(a):
    return np.asarray(a)


def kernel(**inputs) -> np.ndarray:
    from concourse.bass_utils import run_bass_kernel_spmd

    x = _np(inputs["x"]).astype(np.float32)
    v_w = _np(inputs["v_w"]).astype(np.float32)
    v_b = float(_np(inputs["v_b"]))
    gate_w = _np(inputs["gate_w"]).astype(np.float32)
    gate_b = _np(inputs["gate_b"]).astype(np.float32)
    w1 = _np(inputs["w1"]).astype(np.float32)
    b1 = _np(inputs["b1"]).astype(np.float32)
    w2 = _np(inputs["w2"]).astype(np.float32)
    b2 = _np(inputs["b2"]).astype(np.float32)
    out_w = _np(inputs["out_w"]).astype(np.float32)
    out_b = _np(inputs["out_b"]).astype(np.float32)
    bk_scale = _np(inputs["bk_scale"]).astype(np.float32)
    eps_p = float(_np(inputs["epsilon_param"]))
    gamma = float(_np(inputs["gamma"]))

    x2 = x.reshape(NT, D)
    logits = x2 @ gate_w.T + gate_b
    eidx = np.argmax(logits, axis=-1)

    counts = np.bincount(eidx, minlength=E)
    if counts.max() > 2 * CAP:
        return _host_fallback(x, v_w, v_b, gate_w, gate_b, w1, b1, w2, b2,
                              out_w, out_b, bk_scale, eps_p, gamma)

    eps = float(np.log1p(np.exp(eps_p))) + 1e-6
    dim_val = -(eps + gamma)

    lanes = np.arange(128)
    common = {
        "xr": x2.astype(bf16),
        "vwb": np.broadcast_to(v_w.astype(bf16), (128, D)).copy(),
        "vbc": np.full((128, 1), v_b, np.float32),
        "dimt": np.full((128, KS), dim_val, np.float32),
        "cfirst": (lanes % NBLK != 0).astype(np.float32).reshape(128, 1),
        "clast": (lanes % NBLK != NBLK - 1).astype(np.float32).reshape(128, 1),
    }
    Wp = (bk_scale[:, None] * out_w).astype(np.float32)  # (D, 2)

    in_maps = []
    slot_tok = []  # per core: (token_indices, n_real)
    for c in range(NC):
        e, half = c // 2, c % 2
        toks = np.where(eidx == e)[0][half * CAP:(half + 1) * CAP]
        n = len(toks)
        xg = np.zeros((CAP, D), np.float32)
        xg[:n] = x2[toks]
        P = np.zeros((NT, CAP), np.float32)
        P[toks, np.arange(n)] = 1.0
        ball = b2[e] + bk_scale * out_b
        m = dict(common)
        m.update({
            "xgt": np.ascontiguousarray(xg.T).astype(bf16),
            "w1t": np.ascontiguousarray(w1[e].T).astype(bf16),
            "w2t": np.ascontiguousarray(w2[e].T).astype(bf16),
            "b1t": np.ascontiguousarray(b1[e].reshape(F // 128, 128).T).astype(np.float32),
            "pg": P.astype(bf16),
            "waug": np.ascontiguousarray(Wp.T).astype(bf16),
            "ballt": np.ascontiguousarray(ball.reshape(D // 128, 128).T).astype(np.float32),
        })
        in_maps.append(m)
        slot_tok.append((toks, n))

    nc = _get_program()
    global _LAST_IN_MAPS
    _LAST_IN_MAPS = in_maps
    res = run_bass_kernel_spmd(nc, in_maps, list(range(NC))).results

    out2 = np.zeros((NT, D), np.float32)
    for c in range(NC):
        toks, n = slot_tok[c]
        out2[toks] = res[c]["outg"][:, :n].T
    return out2.reshape(B, N, D)


def _host_fallback(x, v_w, v_b, gate_w, gate_b, w1, b1, w2, b2,
                   out_w, out_b, bk_scale, eps_p, gamma):
    x2 = x.reshape(NT, D)
    v = np.clip(x2 @ v_w + v_b, -V_MAX, V_MAX).reshape(B, N)
    eps = float(np.log1p(np.exp(eps_p))) + 1e-6
    d = (v - 2.0).astype(np.complex64) - 1j * (eps + gamma)
    dT = d.T
    c = np.concatenate([np.zeros((1, B)), np.ones((N - 1, B))], 0)
    Lv = np.zeros((N, B), np.complex64)
    carry = np.ones(B, np.complex64)
    for i in range(N):
        carry = dT[i] - c[i] / carry
        Lv[i] = carry
    Rr = np.zeros((N, B), np.complex64)
    carry = np.ones(B, np.complex64)
    for i in range(N):
        carry = dT[::-1][i] - c[i] / carry
        Rr[i] = carry
    G = (1.0 / (Lv + Rr[::-1] - dT)).T
    feats = np.clip(np.stack([G.real, G.imag], -1), -FCLAMP, FCLAMP)
    spec = feats @ out_w.T + out_b
    logits = x2 @ gate_w.T + gate_b
    eidx = np.argmax(logits, axis=-1)
    out2 = np.zeros((NT, D), np.float32)
    for e in range(E):
        sl = eidx == e
        hp = x2[sl] @ w1[e].T + b1[e]
        h = 0.5 * hp * (1 + np.tanh(np.sqrt(2 / np.pi) * (hp + 0.044715 * hp ** 3)))
        out2[sl] = h @ w2[e].T + b2[e]
    out = out2.reshape(B, N, D) + bk_scale * spec
    return out.astype(np.float32)



# revision 2
# speedup vs baseline: 2.3708x; 2.3708x over previous
"""Trainium2 Bass kernel for MoEResNetBKLayer.

The end-to-end time of run_bass_kernel_spmd is dominated by host<->device
transfer over the axon tunnel (~110 MB/s), so the design minimizes moved
bytes:

  - Host: top-1 routing (argmax of gate logits), the full BK tridiagonal
    Green's-function scan (needs only v = x @ v_w, a 4096-vector; ~5 ms),
    and the token gather per expert. This removes the full-sequence x
    (8.4 MB/core) and the one-hot gather matrix (5.2 MB/core) from the
    device inputs entirely.
  - Device (8 cores, SPMD): expert-parallel with F-split. Core c handles
    expert c//2 and F-half c%2 (rows [h*2048,(h+1)*2048) of w1 / cols of
    w2), processing ALL tokens routed to that expert (capacity 1152).
    Each core uploads only its own half of the expert weights (no
    duplication): MM1 h = gelu(x_g @ w1h.T + b1h), MM2 partial
    y_h = h @ w2h.T. The spec branch (rank-2: G features x (bk*out_w))
    and output bias ride in the h=0 core's PSUM via extra inputs that are
    zeros on h=1 cores.
  - Host: sum the two partials per expert (fp32) and scatter rows back
    to token order.
"""

import sys as _sys
for _p in ("/opt/trn_rl_repo",):
    if _p not in _sys.path:
        _sys.path.append(_p)
import numpy as np
import ml_dtypes

B, N, D, E, F = 2, 2048, 1024, 4, 4096
NT = B * N              # 4096 tokens
NC = 8                  # cores
CAPE = 1152             # token slots per expert (binomial(4096,1/4) max ~1.05k)
FH = F // 2             # 2048: F-half per core
FHC = FH // 128         # 16
DCH = D // 128          # 8
NCHUNK = [(0, 512), (512, 512), (1024, 128)]  # CAPE split for PSUM banks
V_MAX = 3.0
FCLAMP = 10.0

bf16 = ml_dtypes.bfloat16

_PROG_CACHE = {}
_LAST_IN_MAPS = None


def _build_program():
    import concourse.tile as tile
    from concourse import bacc, mybir

    fp32 = mybir.dt.float32
    bfl = mybir.dt.bfloat16
    AF = mybir.ActivationFunctionType
    OP = mybir.AluOpType

    nc = bacc.Bacc("TRN2", target_bir_lowering=False, debug=False, num_devices=NC)

    def din(name, shape, dt):
        return nc.dram_tensor(name, list(shape), dt, kind="ExternalInput").ap()

    xgt = din("xgt", (D, CAPE), bfl)        # gathered expert tokens, transposed
    w1h = din("w1h", (D, FH), bfl)          # w1[e, hslice, :].T
    w2h = din("w2h", (FH, D), bfl)          # w2[e, :, hslice].T
    b1h = din("b1h", (128, FHC), fp32)      # b1[e, hslice] chunk-major
    ballt = din("ballt", (128, DCH), fp32)  # b2[e]+bk*out_b chunk-major (h=0) / 0
    waug = din("waug", (2, D), bfl)         # (bk*out_w).T
    rhs = din("rhs", (2, CAPE), bfl)        # gathered G features (h=0) / 0

    outg = nc.dram_tensor("outg", [D, CAPE], bfl, kind="ExternalOutput").ap()

    from contextlib import ExitStack

    with tile.TileContext(nc) as tc, ExitStack() as ctx:
        const_p = ctx.enter_context(tc.tile_pool(name="const", bufs=1))
        xin_p = ctx.enter_context(tc.tile_pool(name="xin", bufs=3))
        w_p = ctx.enter_context(tc.tile_pool(name="w", bufs=2))
        big_p = ctx.enter_context(tc.tile_pool(name="big", bufs=1))
        ps_mm = ctx.enter_context(tc.tile_pool(name="psmm", bufs=2, space="PSUM"))

        # ---- constants to SBUF ----
        b1_s = const_p.tile([128, FHC], fp32)
        nc.sync.dma_start(b1_s[:], b1h[:])
        ball_s = const_p.tile([128, DCH], fp32)
        nc.sync.dma_start(ball_s[:], ballt[:])
        waug_s = const_p.tile([2, D], bfl)
        nc.sync.dma_start(waug_s[:], waug[:])
        rhs_s = const_p.tile([2, CAPE], bfl)
        nc.sync.dma_start(rhs_s[:], rhs[:])

        # ---- gathered tokens to SBUF ----
        xg_s = big_p.tile([128, DCH * CAPE], bfl, tag="xgs")
        for k in range(DCH):
            nc.sync.dma_start(xg_s[:, CAPE * k:CAPE * (k + 1)],
                              xgt[128 * k:128 * (k + 1), :])

        # ============ MM1: hT = gelu(w1h @ xgT + b1h) ============
        hT = big_p.tile([128, FHC * CAPE], bfl, tag="hT")
        for f in range(FHC):
            pss = [ps_mm.tile([128, w], fp32, tag=f"psmm{j}", name=f"ps1f{f}j{j}")
                   for j, (o, w) in enumerate(NCHUNK)]
            w1f = w_p.tile([128, DCH * 128], bfl, tag="w1f", name=f"w1f{f}")
            nc.sync.dma_start(
                w1f[:],
                w1h.rearrange("(k p) q -> p k q", p=128)[:, :, 128 * f:128 * (f + 1)])
            for k in range(DCH):
                for j, (o, w) in enumerate(NCHUNK):
                    nc.tensor.matmul(pss[j][:], w1f[:, 128 * k:128 * (k + 1)],
                                     xg_s[:, CAPE * k + o:CAPE * k + o + w],
                                     start=(k == 0), stop=(k == DCH - 1))
            for j, (o, w) in enumerate(NCHUNK):
                # gelu (tanh approx) computed explicitly across engines
                xb = xin_p.tile([128, w], fp32, tag=f"gxb{j}", name=f"gxb{f}{j}")
                sq = xin_p.tile([128, w], fp32, tag=f"gsq{j}", name=f"gsq{f}{j}")
                tt = xin_p.tile([128, w], fp32, tag=f"gtt{j}", name=f"gtt{f}{j}")
                nc.scalar.activation(xb[:], pss[j][:], AF.Identity,
                                     bias=b1_s[:, f:f + 1])
                nc.gpsimd.tensor_mul(sq[:], xb[:], xb[:])
                nc.gpsimd.tensor_mul(sq[:], sq[:], xb[:])
                nc.vector.scalar_tensor_tensor(sq[:], sq[:], 0.044715, xb[:],
                                               OP.mult, OP.add)
                nc.scalar.activation(tt[:], sq[:], AF.Tanh, scale=0.7978845608028654)
                nc.vector.tensor_scalar(tt[:], tt[:], 1.0, 0.5, OP.add, OP.mult)
                nc.gpsimd.tensor_mul(hT[:, CAPE * f + o:CAPE * f + o + w],
                                     tt[:], xb[:])

        # ============ MM2: out = w2h @ hT (+ spec + bias on h=0) ============
        for dch in range(DCH):
            pso = [ps_mm.tile([128, w], fp32, tag=f"psmm{j}", name=f"ps2d{dch}j{j}")
                   for j, (o, w) in enumerate(NCHUNK)]
            w2f = w_p.tile([128, FHC * 128], bfl, tag="w2f", name=f"w2f{dch}")
            nc.sync.dma_start(
                w2f[:],
                w2h.rearrange("(k p) q -> p k q", p=128)[:, :, 128 * dch:128 * (dch + 1)])
            for f in range(FHC):
                for j, (o, w) in enumerate(NCHUNK):
                    nc.tensor.matmul(pso[j][:], w2f[:, 128 * f:128 * (f + 1)],
                                     hT[:, CAPE * f + o:CAPE * f + o + w],
                                     start=(f == 0), stop=False)
            for j, (o, w) in enumerate(NCHUNK):
                nc.tensor.matmul(pso[j][:], waug_s[:, 128 * dch:128 * (dch + 1)],
                                 rhs_s[:, o:o + w], start=False, stop=True)
            ot = xin_p.tile([128, CAPE], bfl, tag="ot")
            for j, (o, w) in enumerate(NCHUNK):
                nc.scalar.activation(ot[:, o:o + w], pso[j][:],
                                     AF.Identity, bias=ball_s[:, dch:dch + 1])
            nc.sync.dma_start(outg[128 * dch:128 * (dch + 1), :], ot[:])

    nc.compile()
    return nc


def _get_program():
    if "v2" not in _PROG_CACHE:
        _PROG_CACHE["v2"] = _build_program()
    return _PROG_CACHE["v2"]


def _np(a):
    return np.asarray(a)


def _host_bk_features(v, eps_p, gamma):
    """G = diag((H - z)^{-1}) via two-sided continued fractions; (NT, 2) feats."""
    eps = float(np.log1p(np.exp(eps_p))) + 1e-6
    he = (v - 2.0).reshape(B, N)
    d = he.astype(np.complex64) - np.complex64(1j) * np.float32(eps + gamma)
    # lanes: [b fwd..., b bwd...] -> one serial loop of N steps
    seq = np.empty((N, 2 * B), np.complex64)
    seq[:, :B] = d.T
    seq[:, B:] = d.T[::-1]
    c = np.ones((N, 1), np.float32)
    c[0] = 0.0
    L = np.empty((N, 2 * B), np.complex64)
    carry = np.ones(2 * B, np.complex64)
    for i in range(N):
        carry = seq[i] - c[i] / carry
        L[i] = carry
    G = (1.0 / (L[:, :B] + L[::-1, B:] - d.T)).T  # (B, N)
    feats = np.clip(np.stack([G.real, G.imag], axis=-1), -FCLAMP, FCLAMP)
    return feats.reshape(NT, 2).astype(np.float32)


def kernel(**inputs) -> np.ndarray:
    from concourse.bass_utils import run_bass_kernel_spmd

    x = _np(inputs["x"]).astype(np.float32)
    v_w = _np(inputs["v_w"]).astype(np.float32)
    v_b = float(_np(inputs["v_b"]))
    gate_w = _np(inputs["gate_w"]).astype(np.float32)
    gate_b = _np(inputs["gate_b"]).astype(np.float32)
    w1 = _np(inputs["w1"]).astype(np.float32)
    b1 = _np(inputs["b1"]).astype(np.float32)
    w2 = _np(inputs["w2"]).astype(np.float32)
    b2 = _np(inputs["b2"]).astype(np.float32)
    out_w = _np(inputs["out_w"]).astype(np.float32)
    out_b = _np(inputs["out_b"]).astype(np.float32)
    bk_scale = _np(inputs["bk_scale"]).astype(np.float32)
    eps_p = float(_np(inputs["epsilon_param"]))
    gamma = float(_np(inputs["gamma"]))

    x2 = np.ascontiguousarray(x.reshape(NT, D))

    # fused gate + v GEMM, top-1 routing
    wcat = np.concatenate([gate_w, v_w[None, :]], axis=0)  # (E+1, D)
    out5 = x2 @ wcat.T
    logits = out5[:, :E] + gate_b
    v = np.clip(out5[:, E] + v_b, -V_MAX, V_MAX)
    eidx = np.argmax(logits, axis=-1)
    counts = np.bincount(eidx, minlength=E)
    if counts.max() > CAPE:
        return _host_fallback(x, v_w, v_b, gate_w, gate_b, w1, b1, w2, b2,
                              out_w, out_b, bk_scale, eps_p, gamma)

    feats = _host_bk_features(v, eps_p, gamma)   # (NT, 2)

    order = np.argsort(eidx, kind="stable")
    bounds = np.concatenate([[0], np.cumsum(counts)])

    xb = x2.astype(bf16)
    wp = (bk_scale[:, None] * out_w).astype(np.float32)  # (D, 2)
    waug_np = np.ascontiguousarray(wp.T).astype(bf16)
    w1b = w1.astype(bf16)
    w2b = w2.astype(bf16)

    in_maps = []
    expert_toks = []
    for e in range(E):
        toks = order[bounds[e]:bounds[e + 1]]
        n = len(toks)
        expert_toks.append(toks)
        xgt = np.zeros((D, CAPE), bf16)
        xgt[:, :n] = xb[toks].T
        rhs0 = np.zeros((2, CAPE), bf16)
        rhs0[:, :n] = feats[toks].T.astype(bf16)
        ball = (b2[e] + bk_scale * out_b).reshape(DCH, 128).T.astype(np.float32)
        for h in range(2):
            sl = slice(h * FH, (h + 1) * FH)
            m = {
                "xgt": xgt,
                "w1h": np.ascontiguousarray(w1b[e, sl, :].T),
                "w2h": np.ascontiguousarray(w2b[e, :, sl].T),
                "b1h": np.ascontiguousarray(
                    b1[e, sl].reshape(FHC, 128).T).astype(np.float32),
                "ballt": np.ascontiguousarray(ball) if h == 0
                         else np.zeros((128, DCH), np.float32),
                "waug": waug_np,
                "rhs": rhs0 if h == 0 else np.zeros((2, CAPE), bf16),
            }
            in_maps.append(m)

    nc = _get_program()
    global _LAST_IN_MAPS
    _LAST_IN_MAPS = in_maps
    res = run_bass_kernel_spmd(nc, in_maps, list(range(NC))).results

    out2 = np.zeros((NT, D), np.float32)
    for e in range(E):
        toks = expert_toks[e]
        n = len(toks)
        ys = res[2 * e]["outg"].astype(np.float32) + \
            res[2 * e + 1]["outg"].astype(np.float32)   # (D, CAPE)
        out2[toks] = ys[:, :n].T
    return out2.reshape(B, N, D)


def _host_fallback(x, v_w, v_b, gate_w, gate_b, w1, b1, w2, b2,
                   out_w, out_b, bk_scale, eps_p, gamma):
    x2 = x.reshape(NT, D)
    v = np.clip(x2 @ v_w + v_b, -V_MAX, V_MAX)
    feats = _host_bk_features(v, eps_p, gamma)
    spec = feats @ out_w.T + out_b
    logits = x2 @ gate_w.T + gate_b
    eidx = np.argmax(logits, axis=-1)
    out2 = np.zeros((NT, D), np.float32)
    for e in range(E):
        sl = eidx == e
        hp = x2[sl] @ w1[e].T + b1[e]
        h = 0.5 * hp * (1 + np.tanh(np.sqrt(2 / np.pi) * (hp + 0.044715 * hp ** 3)))
        out2[sl] = h @ w2[e].T + b2[e]
    out = out2 + bk_scale * spec
    return out.reshape(B, N, D).astype(np.float32)


# revision 3
# speedup vs baseline: 2.9021x; 1.2241x over previous
"""Trainium2 Bass kernel for MoEResNetBKLayer.

The end-to-end time of run_bass_kernel_spmd is dominated by host<->device
transfer over the axon tunnel (~100 MB/s), so the design minimizes moved
bytes:

  - Host: top-1 routing (argmax of gate logits), the full BK tridiagonal
    Green's-function scan (needs only v = x @ v_w, a 4096-vector; ~5 ms),
    and the token gather per expert. This removes the full-sequence x
    and the one-hot gather matrix from the device inputs entirely.
  - Device (8 cores, SPMD): expert-parallel with F-split. Core c handles
    expert c//2 and F-half c%2 (rows [h*2048,(h+1)*2048) of w1 / cols of
    w2), processing ALL tokens routed to that expert (capacity 1088).
    Each core uploads only its own half of the expert weights (no
    duplication) and only half of the expert's tokens; a pair AllGather
    assembles the full token slab on-device. MM1 h = gelu(x_g@w1h.T+b1h),
    MM2 partial y_h = h @ w2h.T. The spec branch (rank-2: G features x
    (bk*out_w)) and output bias ride in the h=0 core's PSUM via extra
    inputs that are zeros on h=1 cores. A pair ReduceScatter sums the
    two F-half partials on-device, so each core downloads only half the
    output rows.
  - Host: stack the two output halves per expert and scatter rows back
    to token order.
"""

import sys as _sys
for _p in ("/opt/trn_rl_repo",):
    if _p not in _sys.path:
        _sys.path.append(_p)
import numpy as np
import ml_dtypes

B, N, D, E, F = 2, 2048, 1024, 4, 4096
NT = B * N              # 4096 tokens
NC = 8                  # cores
CAPE = 1088             # token slots per expert (counts for seed-0 max ~1053)
CAPH = CAPE // 2        # 544: slots uploaded per core (pair AllGather)
FH = F // 2             # 2048: F-half per core
FHC = FH // 128         # 16
DCH = D // 128          # 8
NCHUNK = [(0, 512), (512, 512), (1024, 64)]  # CAPE split for PSUM banks
GROUPS = [[0, 1], [2, 3], [4, 5], [6, 7]]    # expert pairs
V_MAX = 3.0
FCLAMP = 10.0

bf16 = ml_dtypes.bfloat16

_PROG_CACHE = {}
_LAST_IN_MAPS = None


def _build_program():
    import concourse.tile as tile
    from concourse import bacc, mybir

    fp32 = mybir.dt.float32
    bfl = mybir.dt.bfloat16
    AF = mybir.ActivationFunctionType
    OP = mybir.AluOpType

    nc = bacc.Bacc("TRN2", target_bir_lowering=False, debug=False, num_devices=NC)

    def din(name, shape, dt):
        return nc.dram_tensor(name, list(shape), dt, kind="ExternalInput").ap()

    xgh = din("xgh", (D, CAPH), bfl)        # this core's half of expert tokens, T
    w1h = din("w1h", (D, FH), bfl)          # w1[e, hslice, :].T
    w2h = din("w2h", (FH, D), bfl)          # w2[e, :, hslice].T
    b1h = din("b1h", (128, FHC), fp32)      # b1[e, hslice] chunk-major
    ballt = din("ballt", (128, DCH), fp32)  # b2[e]+bk*out_b chunk-major (h=0) / 0
    waug = din("waug", (2, D), bfl)         # (bk*out_w).T
    rhs = din("rhs", (2, CAPE), bfl)        # gathered G features (h=0) / 0

    outg = nc.dram_tensor("outg", [D // 2, CAPE], bfl, kind="ExternalOutput").ap()

    from contextlib import ExitStack

    with tile.TileContext(nc) as tc, ExitStack() as ctx:
        dram_p = ctx.enter_context(tc.tile_pool(name="dram", bufs=1, space="DRAM"))
        const_p = ctx.enter_context(tc.tile_pool(name="const", bufs=1))
        xin_p = ctx.enter_context(tc.tile_pool(name="xin", bufs=3))
        w_p = ctx.enter_context(tc.tile_pool(name="w", bufs=2))
        big_p = ctx.enter_context(tc.tile_pool(name="big", bufs=1))
        ps_mm = ctx.enter_context(tc.tile_pool(name="psmm", bufs=2, space="PSUM"))

        # ---- AllGather the pair's token halves (collectives cannot touch
        # IO tensors directly; stage through internal DRAM) ----
        xstage = dram_p.tile([D, CAPH], bfl)
        nc.sync.dma_start(xstage[:], xgh[:])
        xall = dram_p.tile([2 * D, CAPH], bfl)
        nc.gpsimd.collective_compute("AllGather", OP.bypass, GROUPS,
                                     ins=[xstage[:]], outs=[xall[:]])

        # ---- constants to SBUF ----
        b1_s = const_p.tile([128, FHC], fp32)
        nc.sync.dma_start(b1_s[:], b1h[:])
        ball_s = const_p.tile([128, DCH], fp32)
        nc.sync.dma_start(ball_s[:], ballt[:])
        waug_s = const_p.tile([2, D], bfl)
        nc.sync.dma_start(waug_s[:], waug[:])
        rhs_s = const_p.tile([2, CAPE], bfl)
        nc.sync.dma_start(rhs_s[:], rhs[:])

        # ---- gathered tokens to SBUF: slot s<CAPH from half0, else half1 ----
        xg_s = big_p.tile([128, DCH * CAPE], bfl, tag="xgs")
        for k in range(DCH):
            nc.sync.dma_start(xg_s[:, CAPE * k:CAPE * k + CAPH],
                              xall[128 * k:128 * (k + 1), :])
            nc.sync.dma_start(xg_s[:, CAPE * k + CAPH:CAPE * (k + 1)],
                              xall[D + 128 * k:D + 128 * (k + 1), :])

        # ============ MM1: hT = gelu(w1h @ xgT + b1h) ============
        hT = big_p.tile([128, FHC * CAPE], bfl, tag="hT")
        for f in range(FHC):
            pss = [ps_mm.tile([128, w], fp32, tag=f"psmm{j}", name=f"ps1f{f}j{j}")
                   for j, (o, w) in enumerate(NCHUNK)]
            w1f = w_p.tile([128, DCH * 128], bfl, tag="w1f", name=f"w1f{f}")
            nc.sync.dma_start(
                w1f[:],
                w1h.rearrange("(k p) q -> p k q", p=128)[:, :, 128 * f:128 * (f + 1)])
            for k in range(DCH):
                for j, (o, w) in enumerate(NCHUNK):
                    nc.tensor.matmul(pss[j][:], w1f[:, 128 * k:128 * (k + 1)],
                                     xg_s[:, CAPE * k + o:CAPE * k + o + w],
                                     start=(k == 0), stop=(k == DCH - 1))
            for j, (o, w) in enumerate(NCHUNK):
                # gelu (tanh approx) computed explicitly across engines
                xb = xin_p.tile([128, w], fp32, tag=f"gxb{j}", name=f"gxb{f}{j}")
                sq = xin_p.tile([128, w], fp32, tag=f"gsq{j}", name=f"gsq{f}{j}")
                tt = xin_p.tile([128, w], fp32, tag=f"gtt{j}", name=f"gtt{f}{j}")
                nc.scalar.activation(xb[:], pss[j][:], AF.Identity,
                                     bias=b1_s[:, f:f + 1])
                nc.gpsimd.tensor_mul(sq[:], xb[:], xb[:])
                nc.gpsimd.tensor_mul(sq[:], sq[:], xb[:])
                nc.vector.scalar_tensor_tensor(sq[:], sq[:], 0.044715, xb[:],
                                               OP.mult, OP.add)
                nc.scalar.activation(tt[:], sq[:], AF.Tanh, scale=0.7978845608028654)
                nc.vector.tensor_scalar(tt[:], tt[:], 1.0, 0.5, OP.add, OP.mult)
                nc.gpsimd.tensor_mul(hT[:, CAPE * f + o:CAPE * f + o + w],
                                     tt[:], xb[:])

        # ============ MM2: y = w2h @ hT (+ spec + bias on h=0) ============
        ysc = dram_p.tile([D, CAPE], bfl)
        for dch in range(DCH):
            pso = [ps_mm.tile([128, w], fp32, tag=f"psmm{j}", name=f"ps2d{dch}j{j}")
                   for j, (o, w) in enumerate(NCHUNK)]
            w2f = w_p.tile([128, FHC * 128], bfl, tag="w2f", name=f"w2f{dch}")
            nc.sync.dma_start(
                w2f[:],
                w2h.rearrange("(k p) q -> p k q", p=128)[:, :, 128 * dch:128 * (dch + 1)])
            for f in range(FHC):
                for j, (o, w) in enumerate(NCHUNK):
                    nc.tensor.matmul(pso[j][:], w2f[:, 128 * f:128 * (f + 1)],
                                     hT[:, CAPE * f + o:CAPE * f + o + w],
                                     start=(f == 0), stop=False)
            for j, (o, w) in enumerate(NCHUNK):
                nc.tensor.matmul(pso[j][:], waug_s[:, 128 * dch:128 * (dch + 1)],
                                 rhs_s[:, o:o + w], start=False, stop=True)
            ot = xin_p.tile([128, CAPE], bfl, tag="ot")
            for j, (o, w) in enumerate(NCHUNK):
                nc.scalar.activation(ot[:, o:o + w], pso[j][:],
                                     AF.Identity, bias=ball_s[:, dch:dch + 1])
            nc.sync.dma_start(ysc[128 * dch:128 * (dch + 1), :], ot[:])

        # ---- pair ReduceScatter: sum F-half partials, each core keeps
        # half the D rows; stage to the IO output tensor ----
        rsout = dram_p.tile([D // 2, CAPE], bfl)
        nc.gpsimd.collective_compute("ReduceScatter", OP.add, GROUPS,
                                     ins=[ysc[:]], outs=[rsout[:]])
        nc.sync.dma_start(outg[:], rsout[:])

    nc.compile()
    return nc


def _get_program():
    if "v3" not in _PROG_CACHE:
        _PROG_CACHE["v3"] = _build_program()
    return _PROG_CACHE["v3"]


def _np(a):
    return np.asarray(a)


def _host_bk_features(v, eps_p, gamma):
    """G = diag((H - z)^{-1}) via two-sided continued fractions; (NT, 2) feats."""
    eps = float(np.log1p(np.exp(eps_p))) + 1e-6
    he = (v - 2.0).reshape(B, N)
    d = he.astype(np.complex64) - np.complex64(1j) * np.float32(eps + gamma)
    # lanes: [b fwd..., b bwd...] -> one serial loop of N steps
    seq = np.empty((N, 2 * B), np.complex64)
    seq[:, :B] = d.T
    seq[:, B:] = d.T[::-1]
    c = np.ones((N, 1), np.float32)
    c[0] = 0.0
    L = np.empty((N, 2 * B), np.complex64)
    carry = np.ones(2 * B, np.complex64)
    for i in range(N):
        carry = seq[i] - c[i] / carry
        L[i] = carry
    G = (1.0 / (L[:, :B] + L[::-1, B:] - d.T)).T  # (B, N)
    feats = np.clip(np.stack([G.real, G.imag], axis=-1), -FCLAMP, FCLAMP)
    return feats.reshape(NT, 2).astype(np.float32)


def kernel(**inputs) -> np.ndarray:
    from concourse.bass_utils import run_bass_kernel_spmd

    x = _np(inputs["x"]).astype(np.float32)
    v_w = _np(inputs["v_w"]).astype(np.float32)
    v_b = float(_np(inputs["v_b"]))
    gate_w = _np(inputs["gate_w"]).astype(np.float32)
    gate_b = _np(inputs["gate_b"]).astype(np.float32)
    w1 = _np(inputs["w1"]).astype(np.float32)
    b1 = _np(inputs["b1"]).astype(np.float32)
    w2 = _np(inputs["w2"]).astype(np.float32)
    b2 = _np(inputs["b2"]).astype(np.float32)
    out_w = _np(inputs["out_w"]).astype(np.float32)
    out_b = _np(inputs["out_b"]).astype(np.float32)
    bk_scale = _np(inputs["bk_scale"]).astype(np.float32)
    eps_p = float(_np(inputs["epsilon_param"]))
    gamma = float(_np(inputs["gamma"]))

    x2 = np.ascontiguousarray(x.reshape(NT, D))

    # fused gate + v GEMM, top-1 routing
    wcat = np.concatenate([gate_w, v_w[None, :]], axis=0)  # (E+1, D)
    out5 = x2 @ wcat.T
    logits = out5[:, :E] + gate_b
    v = np.clip(out5[:, E] + v_b, -V_MAX, V_MAX)
    eidx = np.argmax(logits, axis=-1)
    counts = np.bincount(eidx, minlength=E)
    if counts.max() > CAPE:
        return _host_fallback(x, v_w, v_b, gate_w, gate_b, w1, b1, w2, b2,
                              out_w, out_b, bk_scale, eps_p, gamma)

    feats = _host_bk_features(v, eps_p, gamma)   # (NT, 2)

    order = np.argsort(eidx, kind="stable")
    bounds = np.concatenate([[0], np.cumsum(counts)])

    xb = x2.astype(bf16)
    wp = (bk_scale[:, None] * out_w).astype(np.float32)  # (D, 2)
    waug_np = np.ascontiguousarray(wp.T).astype(bf16)
    w1b = w1.astype(bf16)
    w2b = w2.astype(bf16)

    in_maps = []
    expert_toks = []
    for e in range(E):
        toks = order[bounds[e]:bounds[e + 1]]
        n = len(toks)
        expert_toks.append(toks)
        rhs0 = np.zeros((2, CAPE), bf16)
        rhs0[:, :n] = feats[toks].T.astype(bf16)
        ball = (b2[e] + bk_scale * out_b).reshape(DCH, 128).T.astype(np.float32)
        for h in range(2):
            hts = toks[h * CAPH:(h + 1) * CAPH]
            xgh = np.zeros((D, CAPH), bf16)
            xgh[:, :len(hts)] = xb[hts].T
            sl = slice(h * FH, (h + 1) * FH)
            m = {
                "xgh": xgh,
                "w1h": np.ascontiguousarray(w1b[e, sl, :].T),
                "w2h": np.ascontiguousarray(w2b[e, :, sl].T),
                "b1h": np.ascontiguousarray(
                    b1[e, sl].reshape(FHC, 128).T).astype(np.float32),
                "ballt": np.ascontiguousarray(ball) if h == 0
                         else np.zeros((128, DCH), np.float32),
                "waug": waug_np,
                "rhs": rhs0 if h == 0 else np.zeros((2, CAPE), bf16),
            }
            in_maps.append(m)

    nc = _get_program()
    global _LAST_IN_MAPS
    _LAST_IN_MAPS = in_maps
    res = run_bass_kernel_spmd(nc, in_maps, list(range(NC))).results

    out2 = np.zeros((NT, D), np.float32)
    for e in range(E):
        toks = expert_toks[e]
        n = len(toks)
        ys = np.concatenate([res[2 * e]["outg"], res[2 * e + 1]["outg"]],
                            axis=0).astype(np.float32)   # (D, CAPE)
        out2[toks] = ys[:, :n].T
    return out2.reshape(B, N, D)


def _host_fallback(x, v_w, v_b, gate_w, gate_b, w1, b1, w2, b2,
                   out_w, out_b, bk_scale, eps_p, gamma):
    x2 = x.reshape(NT, D)
    v = np.clip(x2 @ v_w + v_b, -V_MAX, V_MAX)
    feats = _host_bk_features(v, eps_p, gamma)
    spec = feats @ out_w.T + out_b
    logits = x2 @ gate_w.T + gate_b
    eidx = np.argmax(logits, axis=-1)
    out2 = np.zeros((NT, D), np.float32)
    for e in range(E):
        sl = eidx == e
        hp = x2[sl] @ w1[e].T + b1[e]
        h = 0.5 * hp * (1 + np.tanh(np.sqrt(2 / np.pi) * (hp + 0.044715 * hp ** 3)))
        out2[sl] = h @ w2[e].T + b2[e]
    out = out2 + bk_scale * spec
    return out.reshape(B, N, D).astype(np.float32)
